# revision 1
# baseline (speedup 1.0000x reference)
"""Euclidean distance-matrix kernel (retrieval kNN) for Trainium2, 8 cores.

out[g, b, k] = || x[g, b, :] - centroids[g, k, :] ||_2
  via d2 = x2[b] + c2[k] - 2 * x.c   (then sqrt)

Sharding: B axis (8192) split across 8 cores (1024 rows each); centroids
replicated. Everything else is per-core local, no collectives.

Per-core compute structure (G=16 groups, Bs=1024, K=1024, D=64):
  - the -2*x.c term is computed in split-bf16 (x = x_hi + x_lo,
    c = c_hi + c_lo, dropping the lo*lo term) so the PE runs at bf16 rate
    while keeping ~fp32 accuracy. Two matmuls per output tile:
      MM_A: lhsT=[x_hi; x_hi] (128p)   rhs=[-2c_hi; -2c_lo] (128p)
      MM_B: lhsT=[x_lo; 1; 1] (66p)    rhs=[-2c_hi; c2_hi; c2_lo] (66p)
    c2 enters via two bf16 rows (hi+lo split of fp32 c2 keeps precision);
    x2 enters as a per-partition fp32 bias in the ACT sqrt pass.
  - operands need D on partitions; built by bf16 xbar DMA-transposes of
    "mixed" natural-layout tiles so each [128,128] transpose output is
    directly a matmul operand block.
  - ACT computes sqrt(psum + x2) in one pass per [128,1024] tile; DMA out.
"""

import sys

sys.path.insert(0, "/opt/trn_rl_repo")

import numpy as np

import concourse.bacc as bacc
import concourse.tile as tile
from concourse import mybir
from concourse.bass_utils import run_bass_kernel_spmd

G, B, K, D = 16, 8192, 1024, 64
N_CORES = 8
BS = B // N_CORES          # 1024 rows per core
NBT = BS // 128            # 8 b-tiles per group
NKT = K // 128             # 8 k-tiles per group

F32 = mybir.dt.float32
BF16 = mybir.dt.bfloat16
SQRT = mybir.ActivationFunctionType.Sqrt

_cache = {}


def build_nc(repeat: int = 1):
    nc = bacc.Bacc("TRN2", target_bir_lowering=False, debug=False,
                   num_devices=N_CORES)
    x_in = nc.dram_tensor("x", [G, BS, D], F32, kind="ExternalInput").ap()
    c_in = nc.dram_tensor("centroids", [G, K, D], F32, kind="ExternalInput").ap()
    out = nc.dram_tensor("out", [G, BS, K], F32, kind="ExternalOutput").ap()

    with tile.TileContext(nc) as tc:
        with (
            tc.tile_pool(name="prep", bufs=2) as prep,
            tc.tile_pool(name="mix", bufs=2) as mixp,
            tc.tile_pool(name="tp", bufs=2) as tp,
            tc.tile_pool(name="stat", bufs=2) as stat,
            tc.tile_pool(name="outp", bufs=4) as outp,
            tc.tile_pool(name="ps", bufs=3, space="PSUM") as ps,
        ):
            for _ in range(repeat):
                for g in range(G):
                    # ---- natural-layout loads: partition = row-within-tile
                    x_nat = prep.tile([128, NBT, D], F32, tag="x_nat")
                    nc.sync.dma_start(
                        out=x_nat,
                        in_=x_in[g].rearrange("(b p) d -> p b d", p=128),
                    )
                    c_nat = prep.tile([128, NKT, D], F32, tag="c_nat")
                    nc.sync.dma_start(
                        out=c_nat,
                        in_=c_in[g].rearrange("(k p) d -> p k d", p=128),
                    )

                    # ---- x2 per row (fp32, exact) -> ACT bias columns
                    xsq = prep.tile([128, NBT, D], F32, tag="xsq")
                    nc.vector.tensor_mul(xsq, x_nat, x_nat)
                    x2c = stat.tile([128, NBT], F32, tag="x2c")
                    nc.vector.reduce_sum(x2c, xsq, axis=mybir.AxisListType.X)

                    # ---- cs = -2*c (exact scale), c2 = 0.25*sum(cs^2) (exact)
                    cs = prep.tile([128, NKT, D], F32, tag="cs")
                    nc.vector.tensor_scalar_mul(cs, c_nat, -2.0)
                    csq = prep.tile([128, NKT, D], F32, tag="csq")
                    nc.vector.tensor_mul(csq, cs, cs)
                    c2s = stat.tile([128, NKT], F32, tag="c2s")
                    nc.vector.reduce_sum(c2s, csq, axis=mybir.AxisListType.X)
                    c2f = stat.tile([128, NKT], F32, tag="c2f")
                    nc.vector.tensor_scalar_mul(c2f, c2s, 0.25)
                    # c2 hi/lo bf16 split (keeps c2 at ~fp32 precision)
                    c2h = stat.tile([128, NKT], BF16, tag="c2h")
                    nc.vector.tensor_copy(c2h, c2f)
                    c2hup = stat.tile([128, NKT], F32, tag="c2hup")
                    nc.vector.tensor_copy(c2hup, c2h)
                    c2l32 = stat.tile([128, NKT], F32, tag="c2l32")
                    nc.vector.tensor_sub(c2l32, c2f, c2hup)

                    # ---- x hi/lo split, mixed layouts for transposition
                    # xmix slot b: [x_hi | x_hi]  (dup -> MM_A lhsT rows 0..127)
                    xmix = mixp.tile([128, NBT, 2, D], BF16, tag="xmix")
                    nc.vector.tensor_copy(xmix[:, :, 0, :], x_nat)
                    nc.vector.tensor_copy(xmix[:, :, 1, :], x_nat)
                    xup = prep.tile([128, NBT, D], F32, tag="xup")
                    nc.vector.tensor_copy(xup, xmix[:, :, 0, :])
                    xlo32 = prep.tile([128, NBT, D], F32, tag="xlo32")
                    nc.vector.tensor_sub(xlo32, x_nat, xup)
                    # xmix2 slot b: [x_lo | 1 1 | junk]  (-> MM_B lhsT rows 0..65)
                    xmix2 = mixp.tile([128, NBT, 2 * D], BF16, tag="xmix2")
                    nc.vector.tensor_copy(xmix2[:, :, 0:D], xlo32)
                    nc.vector.memset(xmix2[:, :, D:D + 2], 1.0)

                    # ---- c hi/lo split, mixed layouts
                    # cmix slot k: [cs_hi | cs_lo]  (-> MM_A rhs rows 0..127)
                    cmix = mixp.tile([128, NKT, 2, D], BF16, tag="cmix")
                    nc.vector.tensor_copy(cmix[:, :, 0, :], cs)
                    cup = prep.tile([128, NKT, D], F32, tag="cup")
                    nc.vector.tensor_copy(cup, cmix[:, :, 0, :])
                    clo32 = prep.tile([128, NKT, D], F32, tag="clo32")
                    nc.vector.tensor_sub(clo32, cs, cup)
                    nc.vector.tensor_copy(cmix[:, :, 1, :], clo32)
                    # cmix2 slot k: [cs_hi | c2_hi c2_lo | junk] (-> MM_B rhs 0..65)
                    cmix2 = mixp.tile([128, NKT, 2 * D], BF16, tag="cmix2")
                    nc.vector.tensor_copy(cmix2[:, :, 0:D], cmix[:, :, 0, :])
                    nc.vector.tensor_copy(cmix2[:, :, D:D + 1], c2h)
                    nc.vector.tensor_copy(cmix2[:, :, D + 1:D + 2], c2l32)

                    # ---- bf16 xbar transposes -> matmul operand blocks
                    xmixT = tp.tile([128, NBT, 128], BF16, tag="xmixT")
                    xmix2T = tp.tile([128, NBT, 128], BF16, tag="xmix2T")
                    for b in range(NBT):
                        nc.sync.dma_start_transpose(xmixT[:, b, :], xmix[:, b, :, :])
                        nc.sync.dma_start_transpose(xmix2T[:, b, :], xmix2[:, b, :])
                    cmixT = tp.tile([128, NKT, 128], BF16, tag="cmixT")
                    cmix2T = tp.tile([128, NKT, 128], BF16, tag="cmix2T")
                    for k in range(NKT):
                        nc.sync.dma_start_transpose(cmixT[:, k, :], cmix[:, k, :, :])
                        nc.sync.dma_start_transpose(cmix2T[:, k, :], cmix2[:, k, :])

                    # ---- matmuls + sqrt + store, per b-tile
                    for b in range(NBT):
                        psum = ps.tile([128, K], F32, tag="psum")
                        for u in range(2):
                            sl = slice(u * 512, (u + 1) * 512)
                            nc.tensor.matmul(
                                psum[:, sl],
                                lhsT=xmixT[:, b, :],
                                rhs=cmixT[:, 4 * u:4 * (u + 1), :],
                                start=True, stop=False,
                            )
                        for u in range(2):
                            sl = slice(u * 512, (u + 1) * 512)
                            nc.tensor.matmul(
                                psum[:, sl],
                                lhsT=xmix2T[0:66, b, :],
                                rhs=cmix2T[0:66, 4 * u:4 * (u + 1), :],
                                start=False, stop=True,
                            )
                        o_sb = outp.tile([128, K], F32, tag="o_sb")
                        nc.scalar.activation(
                            out=o_sb, in_=psum, func=SQRT,
                            bias=x2c[:, b:b + 1], scale=1.0,
                        )
                        nc.sync.dma_start(
                            out=out[g].rearrange("(b p) k -> p b k", p=128)[:, b, :],
                            in_=o_sb,
                        )
    nc.compile()
    return nc


def get_nc(repeat: int = 1):
    if repeat not in _cache:
        _cache[repeat] = build_nc(repeat)
    return _cache[repeat]


def run(x: np.ndarray, centroids: np.ndarray, repeat: int = 1):
    nc = get_nc(repeat)
    x = np.ascontiguousarray(x, dtype=np.float32)
    centroids = np.ascontiguousarray(centroids, dtype=np.float32)
    in_maps = [
        {"x": x[:, c * BS:(c + 1) * BS, :], "centroids": centroids}
        for c in range(N_CORES)
    ]
    res = run_bass_kernel_spmd(nc, in_maps, list(range(N_CORES)))
    full = np.empty((G, B, K), dtype=np.float32)
    for c in range(N_CORES):
        full[:, c * BS:(c + 1) * BS, :] = res.results[c]["out"]
    return full


def kernel(x: np.ndarray, centroids: np.ndarray) -> np.ndarray:
    return run(x, centroids, repeat=1)
